# revision 11
# baseline (speedup 1.0000x reference)
"""Trainium2 Bass kernel for DilatedNeighborhoodAttention1D (full-attention
reference with periodic 7x7 relative bias), 8-core SPMD.

Sharding: core c -> batch b = c//2, query rows [q0, q0+1024) with
q0 = (c%2)*1024. Each core computes all 8 heads for its query block
against the full 2048 keys of its batch, plus the output projection for
its rows. Concatenation of per-core outputs gives the full (4,2048,256).

Device algorithm per core (all matmul contraction on partitions):
  - qT/kT produced in (channel, seq) layout with the attention scale and
    qkv biases folded in; 7 extra "augmentation" rows carry the rank-7
    factorization of the periodic bias  bias[n,m] = T[n%7 - m%7 + 6]
    so S.T = k_aug.T^T @ q_aug arrives in PSUM with bias included.
  - P = exp(S.T) via ACT directly from PSUM (no max subtraction: |S|<~3).
  - O.T = [v|1]^T @ P accumulated over 16 key tiles; the ones column
    yields the softmax denominators in the same matmul.
  - normalization: 1/r via custom-DVE reciprocal, broadcast to head rows
    via a tiny selector matmul, one tensor-tensor multiply.
  - projection with proj bias as a K=1 ones matmul (v bias pre-folded on
    host into the proj bias since softmax rows sum to 1).
"""

import numpy as np
import ml_dtypes

B, N, C = 4, 2048, 256
H, HD = 8, 32
W = 7
NQ = 1024          # query rows per core
NCORES = 8
SCALE = HD ** -0.5

_BF = ml_dtypes.bfloat16

_nc_cache = {}


def _build_nc():
    import concourse.bass as bass
    import concourse.bacc as bacc
    import concourse.mybir as mybir
    import concourse.tile as tile

    f32 = mybir.dt.float32
    bf16 = mybir.dt.bfloat16
    AF = mybir.ActivationFunctionType
    MS = bass.MemorySpace

    nc = bacc.Bacc(trn_type="TRN2", num_devices=NCORES)

    xt = nc.dram_tensor("xt", [256, 2048], bf16, kind="ExternalInput")
    xtq = nc.dram_tensor("xtq", [256, 1024], bf16, kind="ExternalInput")
    wq = nc.dram_tensor("wq", [256, 256], bf16, kind="ExternalInput")
    wk = nc.dram_tensor("wk", [256, 256], bf16, kind="ExternalInput")
    wv = nc.dram_tensor("wv", [256, 256], bf16, kind="ExternalInput")
    wp = nc.dram_tensor("wp", [256, 256], bf16, kind="ExternalInput")
    bqk = nc.dram_tensor("bqk", [128, 8], f32, kind="ExternalInput")
    augq = nc.dram_tensor("augq", [7, 1024], bf16, kind="ExternalInput")
    augk = nc.dram_tensor("augk", [7, 2048], bf16, kind="ExternalInput")
    bproj = nc.dram_tensor("bproj", [1, 256], bf16, kind="ExternalInput")
    e4 = nc.dram_tensor("e4", [128, 128], f32, kind="ExternalInput")
    y = nc.dram_tensor("y", [1024, 256], f32, kind="ExternalOutput")

    with tile.TileContext(nc) as tc:
        with (
            tc.tile_pool(name="const", bufs=1) as cp,
            tc.tile_pool(name="ppool", bufs=3) as ppool,
            tc.tile_pool(name="ypool", bufs=2) as ypool,
            tc.tile_pool(name="ps_s", bufs=2, space=MS.PSUM) as ps_s,
            tc.tile_pool(name="ps_pv", bufs=2, space=MS.PSUM) as ps_pv,
            tc.tile_pool(name="ps_pr", bufs=2, space=MS.PSUM) as ps_pr,
        ):
            # ---- constant loads ----
            xts = [cp.tile([128, 2048], bf16, tag=f"xts{kc}", name=f"xts{kc}") for kc in range(2)]
            xtqs = [cp.tile([128, 1024], bf16, tag=f"xtqs{kc}", name=f"xtqs{kc}") for kc in range(2)]
            wqs = [cp.tile([128, 256], bf16, tag=f"wqs{kc}", name=f"wqs{kc}") for kc in range(2)]
            wks = [cp.tile([128, 256], bf16, tag=f"wks{kc}", name=f"wks{kc}") for kc in range(2)]
            wvs = [cp.tile([128, 256], bf16, tag=f"wvs{kc}", name=f"wvs{kc}") for kc in range(2)]
            wps = [cp.tile([128, 256], bf16, tag=f"wps{kc}", name=f"wps{kc}") for kc in range(2)]
            for kc in range(2):
                sl = slice(kc * 128, kc * 128 + 128)
                for blk in range(4):
                    cs = slice(blk * 512, blk * 512 + 512)
                    nc.sync.dma_start(xts[kc][:, cs], xt[sl, cs])
                for blk in range(2):
                    cs = slice(blk * 512, blk * 512 + 512)
                    nc.sync.dma_start(xtqs[kc][:, cs], xtq[sl, cs])
                nc.sync.dma_start(wqs[kc][:], wq[sl, :])
                nc.sync.dma_start(wks[kc][:], wk[sl, :])
                nc.sync.dma_start(wvs[kc][:], wv[sl, :])
                nc.sync.dma_start(wps[kc][:], wp[sl, :])
            bqk_sb = cp.tile([128, 8], f32)
            nc.sync.dma_start(bqk_sb[:], bqk[:])
            e4_sb = cp.tile([128, 128], f32)
            nc.sync.dma_start(e4_sb[:], e4[:])
            bproj_sb = cp.tile([1, 256], bf16)
            nc.sync.dma_start(bproj_sb[:], bproj[:])
            ones_sb = cp.tile([1, 128], bf16)
            nc.vector.memset(ones_sb[:], 1.0)

            # per-pair qT/kT tiles: head 2j at partitions 0:32 (+aug 32:39),
            # head 2j+1 at partitions 64:96 (+aug 96:103)
            qts = [cp.tile([128, 1024], bf16, tag=f"qt{j}", name=f"qt{j}") for j in range(4)]
            kts = [cp.tile([128, 2048], bf16, tag=f"kt{j}", name=f"kt{j}") for j in range(4)]
            for j in range(4):
                nc.sync.dma_start(qts[j][32:39, :], augq[:])
                nc.sync.dma_start(qts[j][96:103, :], augq[:])
                nc.sync.dma_start(kts[j][32:39, :], augk[:])
                nc.sync.dma_start(kts[j][96:103, :], augk[:])

            # v in natural layout with a ones column per head: 16 key tiles,
            # each (128 keys, 8 heads x [32 ch | 1]) = (128, 264)
            vS = cp.tile([128, 16 * 264], bf16)
            vview = vS[:].rearrange("p (t h c) -> p t h c", t=16, h=8)
            nc.vector.memset(vview[:, :, :, 32:33], 1.0)

            def prod_qk(j):
                for kind in ("q", "k"):
                    ws = wqs if kind == "q" else wks
                    dst = qts[j] if kind == "q" else kts[j]
                    nblk = 2 if kind == "q" else 4
                    bcol = j if kind == "q" else 4 + j
                    rhs_t = xtqs if kind == "q" else xts
                    for blk in range(nblk):
                        cols = slice(blk * 512, blk * 512 + 512)
                        ps = ps_pr.tile([128, 512], mybir.dt.float32, tag="pr", name="prps")
                        for kc in range(2):
                            for hoff, pos in ((0, None), (64, (0, 64))):
                                h = 2 * j + (0 if hoff == 0 else 1)
                                nc.tensor.matmul(
                                    ps[hoff:hoff + 32, :],
                                    ws[kc][:, h * 32:(h + 1) * 32],
                                    rhs_t[kc][:, cols],
                                    start=(kc == 0), stop=(kc == 1),
                                    tile_position=pos, skip_group_check=True,
                                )
                        for hoff in (0, 64):
                            nc.vector.tensor_scalar_add(
                                dst[hoff:hoff + 32, cols],
                                ps[hoff:hoff + 32, :],
                                bqk_sb[hoff:hoff + 32, bcol:bcol + 1],
                            )

            def prod_v():
                for t in range(16):
                    ps = ps_pr.tile([128, 512], mybir.dt.float32, tag="pr", name="prps")
                    for kc in range(2):
                        nc.tensor.matmul(
                            ps[:, 0:256],
                            xts[kc][:, t * 128:(t + 1) * 128],
                            wvs[kc][:],
                            start=(kc == 0), stop=(kc == 1),
                        )
                    nc.vector.tensor_copy(
                        vview[:, t, :, 0:32],
                        ps[:, 0:256].rearrange("p (h c) -> p h c", h=8),
                    )

            otc = cp.tile([128, 2048], f32)    # raw O.T: [chunk(2) x cht(2) x 512]
            otn = cp.tile([128, 2048], bf16)   # normalized O.T
            # softmax denominators: head h at partition (h%4)*32,
            # free cols (h//4)*1024 + chunk*512 ... +512
            sums = cp.tile([128, 2048], f32)
            rec = cp.tile([128, 2048], f32)
            rscr = cp.tile([128, 2048], f32)
            nc.vector.memset(sums[:], 1.0)

            def attention(j):
                for c in range(2):
                    qa = qts[j][0:39, c * 512:(c + 1) * 512]
                    qb = qts[j][64:103, c * 512:(c + 1) * 512]
                    pv = ps_pv.tile([128, 512], mybir.dt.float32, tag="pv", name="pvps")
                    for t in range(16):
                        sps = ps_s.tile([128, 1024], mybir.dt.float32, tag="s", name="sps")
                        nc.tensor.matmul(
                            sps[:, 0:512],
                            kts[j][0:39, t * 128:(t + 1) * 128], qa,
                            start=True, stop=True,
                        )
                        nc.tensor.matmul(
                            sps[:, 512:1024],
                            kts[j][64:103, t * 128:(t + 1) * 128], qb,
                            start=True, stop=True, tile_position=(64, 0),
                        )
                        P = ppool.tile([128, 1024], bf16, tag="P", name="ptile")
                        nc.scalar.activation(P[:], sps[:], AF.Exp)
                        nc.tensor.matmul(
                            pv[0:33, :],
                            vS[:, t * 264 + (2 * j) * 33: t * 264 + (2 * j) * 33 + 33],
                            P[:, 0:512],
                            start=(t == 0), stop=(t == 15),
                            skip_group_check=True,
                        )
                        nc.tensor.matmul(
                            pv[64:97, :],
                            vS[:, t * 264 + (2 * j + 1) * 33: t * 264 + (2 * j + 1) * 33 + 33],
                            P[:, 512:1024],
                            start=(t == 0), stop=(t == 15), tile_position=(0, 64),
                            skip_group_check=True,
                        )
                    for hoff in (0, 64):
                        h = 2 * j + (0 if hoff == 0 else 1)
                        cht, poff = h // 4, (h % 4) * 32
                        nc.vector.tensor_copy(
                            otc[poff:poff + 32, c * 1024 + cht * 512: c * 1024 + cht * 512 + 512],
                            pv[hoff:hoff + 32, :],
                        )
                        nc.vector.tensor_copy(
                            sums[(h % 4) * 32:(h % 4) * 32 + 1,
                                 (h // 4) * 1024 + c * 512:(h // 4) * 1024 + c * 512 + 512],
                            pv[hoff + 32:hoff + 33, :],
                        )

            def norm_cht(cht):
                # 1/r for this half's heads, broadcast to rows, multiply
                nc.vector.reciprocal_approx_accurate(
                    rec[:, cht * 1024:(cht + 1) * 1024],
                    sums[:, cht * 1024:(cht + 1) * 1024],
                    rscr[:, cht * 1024:(cht + 1) * 1024],
                )
                for c in range(2):
                    rps = ps_pv.tile([128, 512], mybir.dt.float32, tag="pv", name="rps")
                    nc.tensor.matmul(
                        rps[:],
                        e4_sb[:],
                        rec[:, cht * 1024 + c * 512: cht * 1024 + c * 512 + 512],
                        start=True, stop=True,
                    )
                    nc.vector.tensor_mul(
                        otn[:, c * 1024 + cht * 512: c * 1024 + cht * 512 + 512],
                        otc[:, c * 1024 + cht * 512: c * 1024 + cht * 512 + 512],
                        rps[:],
                    )

            # ---- emission: pipeline production of pair j+1 under pair j;
            # half-normalization overlaps the second half's attention ----
            prod_qk(0)
            prod_v()
            attention(0)
            prod_qk(1)
            attention(1)
            prod_qk(2)
            attention(2)
            prod_qk(3)
            attention(3)
            norm_cht(0)
            norm_cht(1)
            for c in range(2):
                for m in range(4):
                    pps = ps_pr.tile([128, 512], mybir.dt.float32, tag="pr", name="pps")
                    nc.tensor.matmul(
                        pps[:, 0:256],
                        otn[:, c * 1024 + m * 128: c * 1024 + m * 128 + 128],
                        wps[0][:], start=True, stop=False,
                    )
                    nc.tensor.matmul(
                        pps[:, 0:256],
                        otn[:, c * 1024 + 512 + m * 128: c * 1024 + 512 + m * 128 + 128],
                        wps[1][:], start=False, stop=False,
                    )
                    nc.tensor.matmul(
                        pps[:, 0:256], ones_sb[:], bproj_sb[:],
                        start=False, stop=True,
                    )
                    ysb = ypool.tile([128, 256], f32, tag="y", name="ysb")
                    nc.vector.tensor_copy(ysb[:], pps[:, 0:256])
                    nc.sync.dma_start(
                        y[c * 512 + m * 128: c * 512 + m * 128 + 128, :], ysb[:]
                    )

    nc.compile()
    return nc


def _host_prep(x, qkv_w, qkv_b, proj_w, proj_b, bias_table):
    """Build the 8 per-core input maps."""
    x = np.asarray(x, np.float32)
    qkv_w = np.asarray(qkv_w, np.float32)
    qkv_b = np.asarray(qkv_b, np.float32)
    proj_w = np.asarray(proj_w, np.float32)
    proj_b = np.asarray(proj_b, np.float32)
    bias_table = np.asarray(bias_table, np.float32)

    wq_h = np.ascontiguousarray((qkv_w[0:256] * SCALE).T.astype(_BF))
    wk_h = np.ascontiguousarray(qkv_w[256:512].T.astype(_BF))
    wv_h = np.ascontiguousarray(qkv_w[512:768].T.astype(_BF))
    wp_h = np.ascontiguousarray(proj_w.T.astype(_BF))

    bq = qkv_b[0:256] * SCALE
    bk = qkv_b[256:512]
    bv = qkv_b[512:768]
    bqk_h = np.zeros((128, 8), np.float32)
    for j in range(4):
        bqk_h[0:32, j] = bq[(2 * j) * 32:(2 * j + 1) * 32]
        bqk_h[64:96, j] = bq[(2 * j + 1) * 32:(2 * j + 2) * 32]
        bqk_h[0:32, 4 + j] = bk[(2 * j) * 32:(2 * j + 1) * 32]
        bqk_h[64:96, 4 + j] = bk[(2 * j + 1) * 32:(2 * j + 2) * 32]

    bproj_h = (proj_b + bv @ proj_w.T).reshape(1, 256).astype(_BF)

    ks = np.arange(2048)
    augk_h = np.zeros((7, 2048), np.float32)
    for a in range(7):
        augk_h[a] = (ks % 7 == a).astype(np.float32)
    augk_h = augk_h.astype(_BF)

    # selector: out[p, n] = rec[32*(p//32), n]
    e4_h = np.zeros((128, 128), np.float32)
    for p in range(128):
        e4_h[(p // 32) * 32, p] = 1.0

    in_maps = []
    for core in range(NCORES):
        b, q0 = core // 2, (core % 2) * NQ
        xt_h = np.ascontiguousarray(x[b].T.astype(_BF))
        xtq_h = np.ascontiguousarray(x[b, q0:q0 + NQ].T.astype(_BF))
        nq = np.arange(q0, q0 + NQ)
        augq_h = np.zeros((7, NQ), np.float32)
        for a in range(7):
            augq_h[a] = bias_table[(nq % 7) - a + 6]
        in_maps.append({
            "xt": xt_h, "xtq": xtq_h,
            "wq": wq_h, "wk": wk_h, "wv": wv_h, "wp": wp_h,
            "bqk": bqk_h, "augq": augq_h.astype(_BF), "augk": augk_h,
            "bproj": bproj_h, "e4": e4_h,
        })
    return in_maps


def kernel(x, qkv_w, qkv_b, proj_w, proj_b, bias_table):
    from concourse.bass_utils import run_bass_kernel_spmd

    if "nc" not in _nc_cache:
        _nc_cache["nc"] = _build_nc()
    nc = _nc_cache["nc"]

    in_maps = _host_prep(x, qkv_w, qkv_b, proj_w, proj_b, bias_table)
    res = run_bass_kernel_spmd(nc, in_maps, list(range(NCORES)))

    out = np.empty((B, N, C), np.float32)
    for core in range(NCORES):
        b, q0 = core // 2, (core % 2) * NQ
        out[b, q0:q0 + NQ] = res.results[core]["y"]
    return out
